# revision 37
# baseline (speedup 1.0000x reference)
"""Causal self-attention on 8 axon-tunneled TRN2 NeuronCores.

Sharding: core c -> (batch b = c//2, head-group g = c%2); host sums the two
head-group partial yT outputs per batch. All matmuls bf16 (1 cyc/row on PE,
f32 PSUM accumulate). The attention phase is ACT(exp)-bound; per-(jc, par)
scores->exp->attn@v stages run in a depth-1 software pipeline (st psum
double-buffered) so ACT stays saturated while PE interleaves the next
score with the previous attn@v. Softmax denominator via a ones-column in v;
1/s broadcast with gpsimd partition_broadcast (keeps PE/PSUM out of the
normalize path); normalize multiplies split across DVE and Pool. Causal
work is trimmed to exact 128-key-block granularity (no block-quantization
waste). PSUM evacuations round-robin across DVE/ACT/Pool."""
import numpy as np

B, T, D = 4, 2048, 1024
NH_LOCAL = 8
HD = 64
CL = 512
P = 128
CC = D // P
TC = T // P
TT = T // 512
NPAIR = 4

_CACHE = {}


def _emit_qkv(nc, tc, mybir, r, aps, qT_sb, kT_sb, v_sb):
    f32 = mybir.dt.float32
    bf16 = mybir.dt.bfloat16
    xT_r, wqT_r, wkT_r, wvT_r = aps
    with tc.tile_pool(name=f"p1x{r}", bufs=1) as p1x, \
         tc.tile_pool(name=f"p1wv{r}", bufs=1) as p1wv, \
         tc.tile_pool(name=f"p1q{r}", bufs=3, space="PSUM") as p1q, \
         tc.tile_pool(name=f"p1ps{r}", bufs=2, space="PSUM") as p1ps:
        xT_sb = p1x.tile([P, CC, T], bf16, tag="xT")
        wv_sb = p1wv.tile([P, CC, CL], bf16, tag="wv")

        with tc.tile_pool(name=f"p1w{r}", bufs=2) as p1w:
            # prefetch first two pairs' weights; only wq0 gates the first
            # matmul group, so it goes out before the xT spans and the rest
            # of the weights follow span 0
            w_slices = []
            for p_i in range(2):
                for w_r, wtag in ((wqT_r, "wq"), (wkT_r, "wk")):
                    w_slices.append(
                        p1w.tile([P, CC, P], bf16, tag=f"{wtag}{p_i}",
                                 name=f"wsl_{wtag}{p_i}"))
            nc.sync.dma_start(w_slices[0][:], wqT_r[:, :, 0:P])
            # xT by 512-token span, one cc-packed DMA per (queue, span), so
            # the first matmul group gates on just wq0 + two span-0 DMAs.
            # Two HWDGE queues (SP + ACT); Pool's software DGE costs ~1us of
            # engine time per DMA, so it issues none.
            for sp in range(TT):
                for par, eng in ((0, nc.scalar), (1, nc.sync)):
                    eng.dma_start(
                        xT_sb[:, par::2, sp * 512:(sp + 1) * 512],
                        xT_r[:, par::2, sp * 512:(sp + 1) * 512])
                if sp == 0:
                    nc.sync.dma_start(w_slices[1][:], wkT_r[:, :, 0:P])
                    nc.sync.dma_start(w_slices[2][:], wqT_r[:, :, P:2 * P])
                    nc.sync.dma_start(w_slices[3][:], wkT_r[:, :, P:2 * P])
            # wv issued here so it never queues behind the ACT evac copies
            nc.scalar.dma_start(wv_sb[:], wvT_r)
            evac = 0
            for p_i in range(NPAIR):
                for wi, (w_r, dst, wtag) in enumerate(
                        ((wqT_r, qT_sb, "wq"), (wkT_r, kT_sb, "wk"))):
                    if p_i < 2:
                        w_sl = w_slices[p_i * 2 + wi]
                    else:
                        w_sl = p1w.tile([P, CC, P], bf16,
                                        tag=f"{wtag}{p_i % 2}")
                        nc.sync.dma_start(
                            w_sl[:], w_r[:, :, p_i * P:(p_i + 1) * P])
                    for half in range(2):
                        pq = p1q.tile([P, 1024], f32, tag="pq")
                        for s5 in range(2):
                            for cc in range(CC):
                                nc.tensor.matmul(
                                    pq[:, s5 * 512:(s5 + 1) * 512],
                                    w_sl[:, cc, :],
                                    xT_sb[:, cc,
                                          half * 1024 + s5 * 512:
                                          half * 1024 + (s5 + 1) * 512],
                                    start=(cc == 0), stop=(cc == CC - 1))
                        dst_ap = dst[:, p_i, half * 1024:(half + 1) * 1024]
                        # Pool/GPSIMD cannot touch PSUM on HW: evacs rotate
                        # DVE and ACT only
                        if evac % 2 == 1:
                            nc.scalar.copy(dst_ap, pq[:])
                        else:
                            nc.vector.tensor_copy(dst_ap, pq[:])
                        evac += 1

        # v = x @ wv^T in [t, c_local] layout (wv DMA already issued before
        # the evac copies entered the ACT queue)
        for t_c in range(TC):
            pv = p1ps.tile([P, CL], f32, tag="pv")
            for cc in range(CC):
                nc.tensor.matmul(
                    pv[:],
                    xT_sb[:, cc, t_c * P:(t_c + 1) * P],
                    wv_sb[:, cc, :],
                    start=(cc == 0), stop=(cc == CC - 1))
            src_ap = pv[:].rearrange("p (h d) -> p h d", h=NH_LOCAL)
            if t_c % 2 == 0:
                nc.vector.tensor_copy(v_sb[:, t_c, :, 0:HD], src_ap)
            else:
                nc.scalar.copy(v_sb[:, t_c, :, 0:HD], src_ap)


def _emit_attention(nc, tc, mybir, r, qT_sb, kT_sb, v_sb, cst_sb, aT_sb):
    """Flat software pipeline over all (head-pair, query-half, key-block)
    stages: scores run 1 stage ahead of exp, attn@v lags 2 stages behind, so
    ACT (the bottleneck) never waits at unit boundaries — the next unit's
    scores are emitted before the previous unit's last attn@v + normalize.
    aT psum accumulators [65, 1024] per par (4 banks); st double-buffered
    (4 banks). Normalize = partition_broadcast of the denominator row +
    divide (per 512-bank, emitted as soon as that bank's accumulation
    stops); it runs entirely off the PE."""
    f32 = mybir.dt.float32
    bf16 = mybir.dt.bfloat16
    EXP = mybir.ActivationFunctionType.Exp
    MULT = mybir.AluOpType.mult
    DIV = mybir.AluOpType.divide
    with tc.tile_pool(name=f"p2{r}", bufs=2) as p2, \
         tc.tile_pool(name=f"p2pt{r}", bufs=8) as p2pt, \
         tc.tile_pool(name=f"p2aps{r}", bufs=1, space="PSUM") as p2aps, \
         tc.tile_pool(name=f"p2sps{r}", bufs=2, space="PSUM") as p2sps:
        units = [(p_i, half) for half in range(2) for p_i in range(NPAIR)]
        stages = []
        for ui, (p_i, half) in enumerate(units):
            for jc in range(8 if half == 0 else 16):
                stages.append((ui, jc))
        n = len(stages)
        aTs_of = {}

        def ctx(ui):
            p_i, half = units[ui]
            return p_i, half * 1024, (half + 1) * 1024, 8 if half == 0 else 16

        def segs_of(ui, jc):
            # exact causal start: key block jc is seen by queries >= 128*jc;
            # split on 512 psum-bank boundaries
            p_i, h0, h1, _ = ctx(ui)
            q0 = max(h0, P * jc)
            return q0, [(b, max(512 * b, q0))
                        for b in range(q0 // 512, h1 // 512)]

        def emit_score(ui, jc, par):
            p_i, h0, h1, _ = ctx(ui)
            q0, segs = segs_of(ui, jc)
            diag = P * jc >= h0
            prow = HD * par
            st = p2sps.tile([P, 1024], f32, tag="st")
            for b, lo in segs:
                lead = diag and lo == q0
                nc.tensor.matmul(
                    st[:, lo - h0:512 * (b + 1) - h0],
                    kT_sb[prow:prow + HD, p_i, jc * P:(jc + 1) * P],
                    qT_sb[prow:prow + HD, p_i, lo:512 * (b + 1)],
                    start=True, stop=not lead)
                if lead:
                    # causal mask folded into the psum group: -1e30 strictly
                    # above the diagonal (keys > query), via mask @ identity
                    nc.tensor.matmul(
                        st[:, q0 - h0:q0 - h0 + P],
                        cst_sb[:, 0, :], cst_sb[:, 1, :],
                        start=False, stop=True)
            return st

        def emit_exp(ui, jc, par, st):
            p_i, h0, h1, _ = ctx(ui)
            q0, _ = segs_of(ui, jc)
            pt = p2pt.tile([P, 1024], bf16, tag="pt")
            nc.scalar.activation(
                pt[:, q0 - h0:1024], st[:, q0 - h0:1024], EXP, scale=0.125)
            return pt

        def emit_norm(ui, b):
            # per-512-bank normalize for both pars, emitted the moment that
            # bank's accumulation stops (first bank mid-unit at jc=4b+3)
            p_i, h0, h1, _ = ctx(ui)
            aTs = aTs_of[ui]
            lo, hi = 512 * b - h0, 512 * (b + 1) - h0
            # 1/s per par -> one row; broadcast to 64 partitions with a
            # K=1 PE matmul (ones column x recip row); DVE evacuates and
            # multiplies. Pool/GPSIMD can't touch PSUM on HW.
            rr = p2.tile([P, 1024], bf16, tag="rr",
                         name=f"rr_{ui}_{b}")
            with nc.allow_low_precision(reason="softmax normalize"):
                for par in range(2):
                    nc.vector.reciprocal(
                        rr[HD:HD + 1, 512 * par:512 * par + 512],
                        aTs[par][HD:HD + 1, lo:hi])
            bc = p2sps.tile([P, 1024], f32, tag="st",
                            name=f"bc_{ui}_{b}")
            for s5 in range(2):  # matmul out must stay within one psum bank
                nc.tensor.matmul(
                    bc[0:HD, 512 * s5:512 * (s5 + 1)],
                    cst_sb[HD:HD + 1, 2, 0:HD],
                    rr[HD:HD + 1, 512 * s5:512 * (s5 + 1)],
                    start=True, stop=True)
            for par in range(2):
                rb = p2.tile([HD, 512], bf16, tag="rb",
                             name=f"rb_{ui}_{par}_{b}")
                nc.vector.tensor_copy(
                    rb[:], bc[0:HD, 512 * par:512 * par + 512])
                if par == 0:
                    nc.vector.tensor_tensor(
                        aT_sb[0:HD, p_i, 512 * b:512 * (b + 1)],
                        aTs[0][0:HD, lo:hi], rb[:], MULT)
                else:
                    t64 = p2.tile([HD, 1024], bf16, tag="t64",
                                  name=f"t64_{ui}_{b}")
                    nc.vector.tensor_tensor(
                        t64[:, lo:hi], aTs[1][0:HD, lo:hi],
                        rb[:], MULT)
                    nc.sync.dma_start(
                        aT_sb[HD:P, p_i, 512 * b:512 * (b + 1)],
                        t64[:, lo:hi])

        def emit_av(ui, jc, par, pt):
            p_i, h0, h1, jc_end = ctx(ui)
            q0, segs = segs_of(ui, jc)
            if ui not in aTs_of:
                aTs_of[ui] = [
                    p2aps.tile([HD + 1, 1024], f32, tag=f"aT{e}",
                               name=f"aT{e}_u{ui}") for e in range(2)]
            h = 2 * p_i + par
            for b, lo in segs:
                nc.tensor.matmul(
                    aTs_of[ui][par][:, lo - h0:512 * (b + 1) - h0],
                    v_sb[:, jc, h, :],
                    pt[:, lo - h0:512 * (b + 1) - h0],
                    start=(jc == 0),
                    stop=(jc == min(4 * b + 3, jc_end - 1)))
            if par == 1:  # both pars' bank-b accumulation stops together
                for b in range(h0 // 512, h1 // 512):
                    if min(4 * b + 3, jc_end - 1) == jc:
                        emit_norm(ui, b)

        sts, pts = {}, {}
        for i in range(n + 3):
            if i < n:
                for par in range(2):
                    sts[(i, par)] = emit_score(*stages[i], par)
            if 1 <= i <= n:
                ui, jc = stages[i - 1]
                for par in range(2):
                    pts[(i - 1, par)] = emit_exp(ui, jc, par,
                                                 sts.pop((i - 1, par)))
            if i >= 3:
                ui, jc = stages[i - 3]
                for par in range(2):
                    emit_av(ui, jc, par, pts.pop((i - 3, par)))


def _emit_out_proj(nc, tc, mybir, r, yT_r, aT_sb, wo_sb):
    f32 = mybir.dt.float32
    with tc.tile_pool(name=f"p3{r}", bufs=4) as p3, \
         tc.tile_pool(name=f"p3ps{r}", bufs=4, space="PSUM") as p3ps:
        k = 0
        for tt in range(TT):  # tt-major: tt 0/1 only need query-half-0 aT
            for fc in range(CC):
                py = p3ps.tile([P, 512], f32, tag="py")
                for cc in range(NPAIR):
                    nc.tensor.matmul(
                        py[:],
                        wo_sb[:, cc, fc * P:(fc + 1) * P],
                        aT_sb[:, cc, tt * 512:(tt + 1) * 512],
                        start=(cc == 0), stop=(cc == NPAIR - 1))
                if fc % 2 == 0:
                    yst = p3.tile([P, 2, 512], f32, tag="yst")
                if k % 2 == 1:
                    nc.scalar.copy(yst[:, fc % 2, :], py[:])
                else:
                    nc.vector.tensor_copy(yst[:, fc % 2, :], py[:])
                if fc % 2 == 1:  # paired store: halves DMA issue + sem count
                    eng = nc.sync if k % 4 == 1 else nc.scalar
                    eng.dma_start(
                        yT_r[:, fc - 1:fc + 1, tt * 512:(tt + 1) * 512],
                        yst[:])
                k += 1


def _build(repeats=1):
    import concourse.bacc as bacc
    import concourse.mybir as mybir
    import concourse.tile as tile
    from contextlib import ExitStack

    f32 = mybir.dt.float32
    bf16 = mybir.dt.bfloat16

    nc = bacc.Bacc("TRN2", target_bir_lowering=False, debug=False)

    xT = nc.dram_tensor("xT", (D, T), bf16, kind="ExternalInput")
    wqT = nc.dram_tensor("wqT", (D, CL), bf16, kind="ExternalInput")
    wkT = nc.dram_tensor("wkT", (D, CL), bf16, kind="ExternalInput")
    wvT = nc.dram_tensor("wvT", (D, CL), bf16, kind="ExternalInput")
    woT = nc.dram_tensor("woT", (CL, D), bf16, kind="ExternalInput")
    cst = nc.dram_tensor("cst", (P, 3 * P), bf16, kind="ExternalInput")
    onesv = nc.dram_tensor("onesv", (P, TC * NH_LOCAL), bf16,
                           kind="ExternalInput")
    yT = nc.dram_tensor("yT", (D, T), f32, kind="ExternalOutput")

    xT_r = xT.ap().rearrange("(o p) t -> p o t", p=P)
    wqT_r = wqT.ap().rearrange("(o p) f -> p o f", p=P)
    wkT_r = wkT.ap().rearrange("(o p) f -> p o f", p=P)
    wvT_r = wvT.ap().rearrange("(o p) f -> p o f", p=P)
    woT_r = woT.ap().rearrange("(o p) f -> p o f", p=P)
    yT_r = yT.ap().rearrange("(o p) t -> p o t", p=P)

    with tile.TileContext(nc) as tc, ExitStack() as outer:
        persist = outer.enter_context(tc.tile_pool(name="persist", bufs=1))
        qT_sb = persist.tile([P, NPAIR, T], bf16, tag="qT")
        kT_sb = persist.tile([P, NPAIR, T], bf16, tag="kT")
        v_sb = persist.tile([P, TC, NH_LOCAL, HD + 1], bf16, tag="v")
        cst_sb = persist.tile([P, 3, P], bf16, tag="cst")
        nc.sync.dma_start(cst_sb[:], cst.ap().rearrange("p (a b) -> p a b", a=3))

        for r in range(repeats):
            # ones column of v via DMA (memset on 16-bit dtypes is not
            # trustworthy on HW)
            nc.sync.dma_start(
                v_sb[:, :, :, HD:HD + 1],
                onesv.ap().rearrange("p (a b o) -> p a b o", a=TC, o=1))
            _emit_qkv(nc, tc, mybir, r, (xT_r, wqT_r, wkT_r, wvT_r),
                      qT_sb, kT_sb, v_sb)
            with tc.tile_pool(name=f"aT{r}", bufs=1) as aTp, \
                 tc.tile_pool(name=f"wo{r}", bufs=1) as wop:
                aT_sb = aTp.tile([P, NPAIR, T], bf16, tag="aT")
                wo_sb = wop.tile([P, NPAIR, D], bf16, tag="wo")
                nc.scalar.dma_start(wo_sb[:], woT_r)
                _emit_attention(nc, tc, mybir, r, qT_sb, kT_sb, v_sb,
                                cst_sb, aT_sb)
                _emit_out_proj(nc, tc, mybir, r, yT_r, aT_sb, wo_sb)

    nc.compile()
    return nc


def _make_in_maps(x, w_qkv, w_out):
    import ml_dtypes
    bf = ml_dtypes.bfloat16
    # cst = [causal -1e30 mask (as matmul lhsT) | identity | ones]
    trin = np.triu(np.full((P, P), -1e30, dtype=np.float32), 1)
    cst = np.concatenate(
        [trin, np.eye(P, dtype=np.float32),
         np.ones((P, P), dtype=np.float32)], axis=1).astype(bf)
    in_maps = []
    for c in range(8):
        b, g = c // 2, c % 2
        sl = slice(CL * g, CL * g + CL)
        in_maps.append({
            "xT": x[b].T.astype(bf),
            "wqT": w_qkv[0 * D:1 * D][sl].T.astype(bf),
            "wkT": w_qkv[1 * D:2 * D][sl].T.astype(bf),
            "wvT": w_qkv[2 * D:3 * D][sl].T.astype(bf),
            "woT": w_out[:, sl].T.astype(bf),
            "cst": cst,
            "onesv": np.ones((P, TC * NH_LOCAL), dtype=bf),
        })
    return in_maps


def kernel(x, w_qkv, w_out):
    from concourse import bass_utils

    if "nc" not in _CACHE:
        _CACHE["nc"] = _build()
    nc = _CACHE["nc"]

    x = np.asarray(x, dtype=np.float32)
    w_qkv = np.asarray(w_qkv, dtype=np.float32)
    w_out = np.asarray(w_out, dtype=np.float32)

    in_maps = _make_in_maps(x, w_qkv, w_out)
    res = bass_utils.run_bass_kernel_spmd(nc, in_maps, core_ids=list(range(8)))
    outs = res.results

    y = np.empty((B, T, D), dtype=np.float32)
    for b in range(B):
        y[b] = (outs[2 * b]["yT"] + outs[2 * b + 1]["yT"]).T
    return y


# revision 40
# speedup vs baseline: 1.9777x; 1.9777x over previous
"""Causal self-attention on 8 axon-tunneled TRN2 NeuronCores.

Sharding: core c -> (batch b = c//2, head-group g = c%2); host sums the two
head-group partial yT outputs per batch. All matmuls bf16 (1 cyc/row on PE,
f32 PSUM accumulate). The attention phase is ACT(exp)-bound; per-(jc, par)
scores->exp->attn@v stages run in a depth-1 software pipeline (st psum
double-buffered) so ACT stays saturated while PE interleaves the next
score with the previous attn@v. Softmax denominator via a ones-column in v;
1/s broadcast with gpsimd partition_broadcast (keeps PE/PSUM out of the
normalize path); normalize multiplies split across DVE and Pool. Causal
work is trimmed to exact 128-key-block granularity (no block-quantization
waste). PSUM evacuations round-robin across DVE/ACT/Pool."""
import numpy as np

B, T, D = 4, 2048, 1024
NH_LOCAL = 8
HD = 64
CL = 512
P = 128
CC = D // P
TC = T // P
TT = T // 512
NPAIR = 4

_CACHE = {}


def _emit_qkv(nc, tc, mybir, r, aps, qT_sb, kT_sb, v_sb):
    f32 = mybir.dt.float32
    bf16 = mybir.dt.bfloat16
    xT_r, wqT_r, wkT_r, wvT_r = aps
    with tc.tile_pool(name=f"p1x{r}", bufs=1) as p1x, \
         tc.tile_pool(name=f"p1wv{r}", bufs=1) as p1wv, \
         tc.tile_pool(name=f"p1q{r}", bufs=3, space="PSUM") as p1q, \
         tc.tile_pool(name=f"p1ps{r}", bufs=2, space="PSUM") as p1ps:
        xT_sb = p1x.tile([P, CC, T], bf16, tag="xT")
        wv_sb = p1wv.tile([P, CC, CL], bf16, tag="wv")

        with tc.tile_pool(name=f"p1w{r}", bufs=2) as p1w:
            # prefetch first two pairs' weights; only wq0 gates the first
            # matmul group, so it goes out before the xT spans and the rest
            # of the weights follow span 0
            w_slices = []
            for p_i in range(2):
                for w_r, wtag in ((wqT_r, "wq"), (wkT_r, "wk")):
                    w_slices.append(
                        p1w.tile([P, CC, P], bf16, tag=f"{wtag}{p_i}",
                                 name=f"wsl_{wtag}{p_i}"))
            nc.sync.dma_start(w_slices[0][:], wqT_r[:, :, 0:P])
            # xT by 512-token span, one cc-packed DMA per (queue, span), so
            # the first matmul group gates on just wq0 + two span-0 DMAs.
            # Two HWDGE queues (SP + ACT); Pool's software DGE costs ~1us of
            # engine time per DMA, so it issues none.
            for sp in range(TT):
                for par, eng in ((0, nc.scalar), (1, nc.sync)):
                    eng.dma_start(
                        xT_sb[:, par::2, sp * 512:(sp + 1) * 512],
                        xT_r[:, par::2, sp * 512:(sp + 1) * 512])
            # all xT spans first: pair0-q half1 needs span 3 by ~8us, while
            # wk0/wq1/wk1 aren't consumed until ~11/18/25us
            nc.sync.dma_start(w_slices[1][:], wkT_r[:, :, 0:P])
            nc.sync.dma_start(w_slices[2][:], wqT_r[:, :, P:2 * P])
            nc.sync.dma_start(w_slices[3][:], wkT_r[:, :, P:2 * P])
            # wv issued here so it never queues behind the ACT evac copies
            nc.scalar.dma_start(wv_sb[:], wvT_r)
            evac = 0
            for p_i in range(NPAIR):
                for wi, (w_r, dst, wtag) in enumerate(
                        ((wqT_r, qT_sb, "wq"), (wkT_r, kT_sb, "wk"))):
                    if p_i < 2:
                        w_sl = w_slices[p_i * 2 + wi]
                    else:
                        w_sl = p1w.tile([P, CC, P], bf16,
                                        tag=f"{wtag}{p_i % 2}")
                        nc.sync.dma_start(
                            w_sl[:], w_r[:, :, p_i * P:(p_i + 1) * P])
                    for half in range(2):
                        pq = p1q.tile([P, 1024], f32, tag="pq")
                        for s5 in range(2):
                            for cc in range(CC):
                                nc.tensor.matmul(
                                    pq[:, s5 * 512:(s5 + 1) * 512],
                                    w_sl[:, cc, :],
                                    xT_sb[:, cc,
                                          half * 1024 + s5 * 512:
                                          half * 1024 + (s5 + 1) * 512],
                                    start=(cc == 0), stop=(cc == CC - 1))
                        dst_ap = dst[:, p_i, half * 1024:(half + 1) * 1024]
                        # Pool/GPSIMD cannot touch PSUM on HW: evacs rotate
                        # DVE and ACT only
                        if evac % 2 == 1:
                            nc.scalar.copy(dst_ap, pq[:])
                        else:
                            nc.vector.tensor_copy(dst_ap, pq[:])
                        evac += 1

        # v = x @ wv^T in [t, c_local] layout (wv DMA already issued before
        # the evac copies entered the ACT queue)
        for t_c in range(TC):
            pv = p1ps.tile([P, CL], f32, tag="pv")
            for cc in range(CC):
                nc.tensor.matmul(
                    pv[:],
                    xT_sb[:, cc, t_c * P:(t_c + 1) * P],
                    wv_sb[:, cc, :],
                    start=(cc == 0), stop=(cc == CC - 1))
            src_ap = pv[:].rearrange("p (h d) -> p h d", h=NH_LOCAL)
            if t_c % 2 == 0:
                nc.vector.tensor_copy(v_sb[:, t_c, :, 0:HD], src_ap)
            else:
                nc.scalar.copy(v_sb[:, t_c, :, 0:HD], src_ap)


def _emit_attention(nc, tc, mybir, r, qT_sb, kT_sb, v_sb, cst_sb, aT_sb):
    """Flat software pipeline over all (head-pair, query-half, key-block)
    stages: scores run 1 stage ahead of exp, attn@v lags 2 stages behind, so
    ACT (the bottleneck) never waits at unit boundaries — the next unit's
    scores are emitted before the previous unit's last attn@v + normalize.
    aT psum accumulators [65, 1024] per par (4 banks); st double-buffered
    (4 banks). Normalize = partition_broadcast of the denominator row +
    divide (per 512-bank, emitted as soon as that bank's accumulation
    stops); it runs entirely off the PE."""
    f32 = mybir.dt.float32
    bf16 = mybir.dt.bfloat16
    EXP = mybir.ActivationFunctionType.Exp
    MULT = mybir.AluOpType.mult
    DIV = mybir.AluOpType.divide
    with tc.tile_pool(name=f"p2{r}", bufs=2) as p2, \
         tc.tile_pool(name=f"p2pt{r}", bufs=8) as p2pt, \
         tc.tile_pool(name=f"p2aps{r}", bufs=1, space="PSUM") as p2aps, \
         tc.tile_pool(name=f"p2sps{r}", bufs=2, space="PSUM") as p2sps:
        units = [(p_i, half) for half in range(2) for p_i in range(NPAIR)]
        stages = []
        for ui, (p_i, half) in enumerate(units):
            for jc in range(8 if half == 0 else 16):
                stages.append((ui, jc))
        n = len(stages)
        aTs_of = {}

        def ctx(ui):
            p_i, half = units[ui]
            return p_i, half * 1024, (half + 1) * 1024, 8 if half == 0 else 16

        def segs_of(ui, jc):
            # exact causal start: key block jc is seen by queries >= 128*jc;
            # split on 512 psum-bank boundaries
            p_i, h0, h1, _ = ctx(ui)
            q0 = max(h0, P * jc)
            return q0, [(b, max(512 * b, q0))
                        for b in range(q0 // 512, h1 // 512)]

        def emit_score(ui, jc, par):
            p_i, h0, h1, _ = ctx(ui)
            q0, segs = segs_of(ui, jc)
            diag = P * jc >= h0
            prow = HD * par
            st = p2sps.tile([P, 1024], f32, tag="st")
            for b, lo in segs:
                lead = diag and lo == q0
                nc.tensor.matmul(
                    st[:, lo - h0:512 * (b + 1) - h0],
                    kT_sb[prow:prow + HD, p_i, jc * P:(jc + 1) * P],
                    qT_sb[prow:prow + HD, p_i, lo:512 * (b + 1)],
                    start=True, stop=not lead)
                if lead:
                    # causal mask folded into the psum group: -1e30 strictly
                    # above the diagonal (keys > query), via mask @ identity
                    nc.tensor.matmul(
                        st[:, q0 - h0:q0 - h0 + P],
                        cst_sb[:, 0, :], cst_sb[:, 1, :],
                        start=False, stop=True)
            return st

        def emit_exp(ui, jc, par, st):
            p_i, h0, h1, _ = ctx(ui)
            q0, _ = segs_of(ui, jc)
            pt = p2pt.tile([P, 1024], bf16, tag="pt")
            nc.scalar.activation(
                pt[:, q0 - h0:1024], st[:, q0 - h0:1024], EXP, scale=0.125)
            return pt

        def emit_norm(ui, b):
            # per-512-bank normalize for both pars, emitted the moment that
            # bank's accumulation stops (first bank mid-unit at jc=4b+3)
            p_i, h0, h1, _ = ctx(ui)
            aTs = aTs_of[ui]
            lo, hi = 512 * b - h0, 512 * (b + 1) - h0
            # 1/s per par -> one row; broadcast to 64 partitions with a
            # K=1 PE matmul (ones column x recip row); DVE evacuates and
            # multiplies. Pool/GPSIMD can't touch PSUM on HW.
            rr = p2.tile([P, 1024], bf16, tag="rr",
                         name=f"rr_{ui}_{b}")
            with nc.allow_low_precision(reason="softmax normalize"):
                for par in range(2):
                    nc.vector.reciprocal(
                        rr[HD:HD + 1, 512 * par:512 * par + 512],
                        aTs[par][HD:HD + 1, lo:hi])
            bc = p2sps.tile([P, 1024], f32, tag="st",
                            name=f"bc_{ui}_{b}")
            for s5 in range(2):  # matmul out must stay within one psum bank
                nc.tensor.matmul(
                    bc[0:HD, 512 * s5:512 * (s5 + 1)],
                    cst_sb[HD:HD + 1, 2, 0:HD],
                    rr[HD:HD + 1, 512 * s5:512 * (s5 + 1)],
                    start=True, stop=True)
            # one op can read only one PSUM operand: evacuate bc for
            # both pars in a single copy, then multiply per par
            rb = p2.tile([HD, 1024], bf16, tag="rb",
                         name=f"rb_{ui}_{b}")
            nc.vector.tensor_copy(rb[:], bc[0:HD, :])
            nc.vector.tensor_tensor(
                aT_sb[0:HD, p_i, 512 * b:512 * (b + 1)],
                aTs[0][0:HD, lo:hi], rb[:, 0:512], MULT)
            t64 = p2.tile([HD, 1024], bf16, tag="t64",
                          name=f"t64_{ui}_{b}")
            nc.vector.tensor_tensor(
                t64[:, lo:hi], aTs[1][0:HD, lo:hi],
                rb[:, 512:1024], MULT)
            nc.sync.dma_start(
                aT_sb[HD:P, p_i, 512 * b:512 * (b + 1)],
                t64[:, lo:hi])

        def emit_av(ui, jc, par, pt):
            p_i, h0, h1, jc_end = ctx(ui)
            q0, segs = segs_of(ui, jc)
            if ui not in aTs_of:
                aTs_of[ui] = [
                    p2aps.tile([HD + 1, 1024], f32, tag=f"aT{e}",
                               name=f"aT{e}_u{ui}") for e in range(2)]
            h = 2 * p_i + par
            for b, lo in segs:
                nc.tensor.matmul(
                    aTs_of[ui][par][:, lo - h0:512 * (b + 1) - h0],
                    v_sb[:, jc, h, :],
                    pt[:, lo - h0:512 * (b + 1) - h0],
                    start=(jc == 0),
                    stop=(jc == min(4 * b + 3, jc_end - 1)))
            if par == 1:  # both pars' bank-b accumulation stops together
                for b in range(h0 // 512, h1 // 512):
                    if min(4 * b + 3, jc_end - 1) == jc:
                        emit_norm(ui, b)

        sts, pts = {}, {}
        for i in range(n + 3):
            if i < n:
                for par in range(2):
                    sts[(i, par)] = emit_score(*stages[i], par)
            if 1 <= i <= n:
                ui, jc = stages[i - 1]
                for par in range(2):
                    pts[(i - 1, par)] = emit_exp(ui, jc, par,
                                                 sts.pop((i - 1, par)))
            if i >= 3:
                ui, jc = stages[i - 3]
                for par in range(2):
                    emit_av(ui, jc, par, pts.pop((i - 3, par)))


def _emit_out_proj(nc, tc, mybir, r, yT_r, aT_sb, wo_sb):
    f32 = mybir.dt.float32
    with tc.tile_pool(name=f"p3{r}", bufs=4) as p3, \
         tc.tile_pool(name=f"p3ps{r}", bufs=4, space="PSUM") as p3ps:
        k = 0
        for tt in range(TT):  # tt-major: tt 0/1 only need query-half-0 aT
            for fc in range(CC):
                py = p3ps.tile([P, 512], f32, tag="py")
                for cc in range(NPAIR):
                    nc.tensor.matmul(
                        py[:],
                        wo_sb[:, cc, fc * P:(fc + 1) * P],
                        aT_sb[:, cc, tt * 512:(tt + 1) * 512],
                        start=(cc == 0), stop=(cc == NPAIR - 1))
                if fc % 2 == 0:
                    yst = p3.tile([P, 2, 512], f32, tag="yst")
                if k % 2 == 1:
                    nc.scalar.copy(yst[:, fc % 2, :], py[:])
                else:
                    nc.vector.tensor_copy(yst[:, fc % 2, :], py[:])
                if tt == TT - 1:  # unpaired final stores: shorter drain
                    eng = nc.sync if k % 2 == 1 else nc.scalar
                    eng.dma_start(
                        yT_r[:, fc, tt * 512:(tt + 1) * 512],
                        yst[:, fc % 2, :])
                elif fc % 2 == 1:  # paired store: halves DMA issue + sems
                    eng = nc.sync if k % 4 == 1 else nc.scalar
                    eng.dma_start(
                        yT_r[:, fc - 1:fc + 1, tt * 512:(tt + 1) * 512],
                        yst[:])
                k += 1


def _build(repeats=1):
    import concourse.bacc as bacc
    import concourse.mybir as mybir
    import concourse.tile as tile
    from contextlib import ExitStack

    f32 = mybir.dt.float32
    bf16 = mybir.dt.bfloat16

    nc = bacc.Bacc("TRN2", target_bir_lowering=False, debug=False)

    xT = nc.dram_tensor("xT", (D, T), bf16, kind="ExternalInput")
    wqT = nc.dram_tensor("wqT", (D, CL), bf16, kind="ExternalInput")
    wkT = nc.dram_tensor("wkT", (D, CL), bf16, kind="ExternalInput")
    wvT = nc.dram_tensor("wvT", (D, CL), bf16, kind="ExternalInput")
    woT = nc.dram_tensor("woT", (CL, D), bf16, kind="ExternalInput")
    cst = nc.dram_tensor("cst", (P, 3 * P), bf16, kind="ExternalInput")
    yT = nc.dram_tensor("yT", (D, T), f32, kind="ExternalOutput")

    xT_r = xT.ap().rearrange("(o p) t -> p o t", p=P)
    wqT_r = wqT.ap().rearrange("(o p) f -> p o f", p=P)
    wkT_r = wkT.ap().rearrange("(o p) f -> p o f", p=P)
    wvT_r = wvT.ap().rearrange("(o p) f -> p o f", p=P)
    woT_r = woT.ap().rearrange("(o p) f -> p o f", p=P)
    yT_r = yT.ap().rearrange("(o p) t -> p o t", p=P)

    with tile.TileContext(nc) as tc, ExitStack() as outer:
        persist = outer.enter_context(tc.tile_pool(name="persist", bufs=1))
        qT_sb = persist.tile([P, NPAIR, T], bf16, tag="qT")
        kT_sb = persist.tile([P, NPAIR, T], bf16, tag="kT")
        v_sb = persist.tile([P, TC, NH_LOCAL, HD + 1], bf16, tag="v")
        cst_sb = persist.tile([P, 3, P], bf16, tag="cst")
        nc.sync.dma_start(cst_sb[:], cst.ap().rearrange("p (a b) -> p a b", a=3))

        for r in range(repeats):
            # ones column of v from the SBUF constant block (strided engine
            # write; a DMA here would shatter into 2-byte descriptors)
            nc.gpsimd.tensor_copy(
                v_sb[:, :, :, HD:HD + 1],
                cst_sb[:, 2, :].rearrange("p (a b) -> p a b", a=TC))
            _emit_qkv(nc, tc, mybir, r, (xT_r, wqT_r, wkT_r, wvT_r),
                      qT_sb, kT_sb, v_sb)
            with tc.tile_pool(name=f"aT{r}", bufs=1) as aTp, \
                 tc.tile_pool(name=f"wo{r}", bufs=1) as wop:
                aT_sb = aTp.tile([P, NPAIR, T], bf16, tag="aT")
                wo_sb = wop.tile([P, NPAIR, D], bf16, tag="wo")
                nc.scalar.dma_start(wo_sb[:], woT_r)
                _emit_attention(nc, tc, mybir, r, qT_sb, kT_sb, v_sb,
                                cst_sb, aT_sb)
                _emit_out_proj(nc, tc, mybir, r, yT_r, aT_sb, wo_sb)

    nc.compile()
    return nc


def _make_in_maps(x, w_qkv, w_out):
    import ml_dtypes
    bf = ml_dtypes.bfloat16
    # cst = [causal -1e30 mask (as matmul lhsT) | identity | ones]
    trin = np.triu(np.full((P, P), -1e30, dtype=np.float32), 1)
    cst = np.concatenate(
        [trin, np.eye(P, dtype=np.float32),
         np.ones((P, P), dtype=np.float32)], axis=1).astype(bf)
    in_maps = []
    for c in range(8):
        b, g = c // 2, c % 2
        sl = slice(CL * g, CL * g + CL)
        in_maps.append({
            "xT": x[b].T.astype(bf),
            "wqT": w_qkv[0 * D:1 * D][sl].T.astype(bf),
            "wkT": w_qkv[1 * D:2 * D][sl].T.astype(bf),
            "wvT": w_qkv[2 * D:3 * D][sl].T.astype(bf),
            "woT": w_out[:, sl].T.astype(bf),
            "cst": cst,
        })
    return in_maps


def kernel(x, w_qkv, w_out):
    from concourse import bass_utils

    if "nc" not in _CACHE:
        _CACHE["nc"] = _build()
    nc = _CACHE["nc"]

    x = np.asarray(x, dtype=np.float32)
    w_qkv = np.asarray(w_qkv, dtype=np.float32)
    w_out = np.asarray(w_out, dtype=np.float32)

    in_maps = _make_in_maps(x, w_qkv, w_out)
    res = bass_utils.run_bass_kernel_spmd(nc, in_maps, core_ids=list(range(8)))
    outs = res.results

    y = np.empty((B, T, D), dtype=np.float32)
    for b in range(B):
        y[b] = (outs[2 * b]["yT"] + outs[2 * b + 1]["yT"]).T
    return y
